# revision 1
# baseline (speedup 1.0000x reference)
"""
Trainium2 Bass kernel for CondConv mask head (CondInst-style dynamic mask head).

Computation (for the fixed problem size):
  mask_feats (2, 8, 136, 200), 128 instances with per-instance 169 params
  -> per-instance 3-layer 1x1 convs over [rel_coords(2); feats(8)] -> (128,1,136,200)
  -> aligned_bilinear x2 upsample -> sigmoid -> (128, 1, 272, 400)

Strategy (8 NeuronCores, 16 instances per core):
  * Host folds the per-instance rel-coordinate channels into a shared 19-row
    spatial matrix Z = [locs_x; locs_y; ones; feats_im0(8); feats_im1(8)] and a
    per-(instance,outchan) lhsT A0T (19, 128); the c0 constant term rides on the
    ones-row.  Layers 1/2 become block-diagonal lhsTs (128,128)/(128,16).
    All matmuls run in float32r (1 PE cycle/row, ~tf32 precision).
  * The image is processed in three W-phases (cols 0:72 / 72:136 / 136:200,
    reordered on host) so each phase's upsample/sigmoid/output-DMA overlaps
    the next phase's conv.  The conv units are software-pipelined in emission
    order (mm0(i) | mm1(i-1) | mm2(i-2)) so the in-order PE queue never blocks
    behind a same-unit eviction, and each phase's upsample emission is deferred
    a few units to avoid head-of-line blocking in the ScalarE queue.
  * PSUM evictions (relu+bias) are whole FD-1024 instructions, alternating
    ScalarE / VectorE (47%/53%).
  * Logits y2 (16, per-phase, incl. a front pad duplicating row 0) are
    re-partitioned by SBUF->SBUF DMAs with overlapping-window access patterns
    into a blocked layout (128 partitions = 16 inst x 8 row-blocks, 17 rows +
    1 halo row each) so the x2 bilinear upsample uses all 128 lanes; the
    upsample runs in row-bands (three on the exposed last phase) so its
    DMA->A/B->C->sigmoid->out-DMA chain pipelines against itself.  A band's
    top boundary wout row is sigmoided/output by the NEXT band, which still
    reads it pre-activation.
  * aligned_bilinear(t, 2) per axis: out[0]=in[0]; out[2k+1]=in[k];
    out[2k]=avg(in[k-1],in[k]).  Wout rows are stored as 2x values and row
    sums as 4x, so each band needs only two contiguous sigmoid instructions
    with the 0.5/0.25 factors folded into the activation's scale operand.
"""

import os
import numpy as np

CH = 8
CIN = 8
N_IMG, H, W = 2, 136, 200
HW = H * W                      # 27200
N_INST = 128
N_CORES = 8
IPC = 16                        # instances per core
FACTOR = 2
OH, OW = H * FACTOR, W * FACTOR  # 272, 400
BLK = 8                         # row-blocks per instance
RPB = H // BLK                  # 17 in-rows per block
ORPB = RPB * FACTOR             # 34 out-rows per block
K0 = 3 + N_IMG * CIN            # 19 contraction rows for layer 0

PHW = [72, 64, 64]              # W-phase widths (small final tail)
NPH = len(PHW)
PHOFF = [0, 72, 136]            # column offset of each phase
PHWMAX = 72
# z-chunk widths per phase (starter + rest, sized <= ~19KB/partition)
PHCHUNKS = [[512, 4608, 4672], [512, 4608, 3584], [512, 4608, 3584]]

LAST_EXEC_TIME_NS = None
_CACHE = {}


def _units(spatial):
    """Pair units (off, wa, wb) covering `spatial` columns."""
    units = []
    off = 0
    while spatial - off > 1024:
        units.append((off, 512, 512))
        off += 1024
    r = spatial - off
    units.append((off, r, 0) if r <= 512 else (off, 512, r - 512))
    return units


def _build_program():
    import concourse.bass as bass
    import concourse.bacc as bacc
    import concourse.tile as tile
    from concourse import mybir
    from contextlib import ExitStack

    f32 = mybir.dt.float32
    f32r = mybir.dt.float32r
    Alu = mybir.AluOpType
    Act = mybir.ActivationFunctionType

    nc = bacc.Bacc("TRN2", target_bir_lowering=False, debug=False)

    zd = nc.dram_tensor("z_in", [K0, HW], f32r, kind="ExternalInput").ap()
    a0d = nc.dram_tensor("a0t_in", [K0, 128], f32r, kind="ExternalInput").ap()
    wpd = nc.dram_tensor("wpack_in", [128, 146], f32r, kind="ExternalInput").ap()
    outd = nc.dram_tensor("out", [IPC, OH, OW], f32, kind="ExternalOutput").ap()

    with tile.TileContext(nc) as tc, ExitStack() as ctx:
        consts = ctx.enter_context(tc.tile_pool(name="consts", bufs=1))
        a0t = consts.tile([K0, 128], f32r)
        wp = consts.tile([128, 146], f32r)
        nc.sync.dma_start(a0t[:], a0d)
        nc.sync.dma_start(wp[:], wpd)
        a0r = a0t[:]
        w1r = wp[:, 0:128]
        w2r = wp[:, 128:144]
        b1ap = wp[:, 144:145].bitcast(f32)
        b2ap = wp[0:16, 145:146].bitcast(f32)

        zs = ctx.enter_context(tc.tile_pool(name="zs", bufs=2))
        zc = ctx.enter_context(tc.tile_pool(name="zc", bufs=2))
        y0p = ctx.enter_context(tc.tile_pool(name="y0p", bufs=3))
        y1p = ctx.enter_context(tc.tile_pool(name="y1p", bufs=4))
        p0 = ctx.enter_context(tc.tile_pool(name="p0", bufs=2, space="PSUM"))
        p12 = ctx.enter_context(tc.tile_pool(name="p12", bufs=2, space="PSUM"))
        y2p = ctx.enter_context(tc.tile_pool(name="y2p", bufs=2))
        fxp = ctx.enter_context(tc.tile_pool(name="fxp", bufs=2))
        hinp = ctx.enter_context(tc.tile_pool(name="hinp", bufs=2))

        ev_ct = [0]

        def evict(dst, src, bias_ap, relu):
            on_act = (ev_ct[0] * 45) % 100 < 45
            ev_ct[0] += 1
            if on_act:
                if relu:
                    if bias_ap is None:
                        nc.scalar.activation(dst, src, Act.Relu)
                    else:
                        nc.scalar.activation(dst, src, Act.Relu, bias=bias_ap)
                else:
                    nc.scalar.activation(dst, src, Act.Identity, bias=bias_ap)
            else:
                if relu:
                    if bias_ap is None:
                        nc.vector.tensor_scalar(dst, src, 0.0, None, Alu.max)
                    else:
                        nc.vector.tensor_scalar(dst, src, bias_ap, 0.0,
                                                Alu.add, Alu.max)
                else:
                    nc.vector.tensor_scalar(dst, src, bias_ap, None, Alu.add)

        prev_hin3 = None

        # flat unit list across phases, software-pipelined (depth 2) so the
        # PE queue never blocks behind an eviction of the same unit
        flat = []
        for h in range(NPH):
            for u in _units(H * PHW[h]):
                flat.append((h, u))

        phase_state = {}
        zoff = 0
        upsample_emits = []

        def start_phase(h):
            nonlocal zoff
            WHp = PHW[h]
            SP = H * WHp
            zts, chunk_offs = [], []
            co = 0
            for ci, cw in enumerate(PHCHUNKS[h]):
                pool_ = zs if ci == 0 else zc
                zt = pool_.tile([K0, cw], f32r, tag="z0" if ci == 0 else "z")
                nc.scalar.dma_start(zt[:], zd[:, zoff + co: zoff + co + cw])
                zts.append(zt)
                chunk_offs.append(co)
                co += cw
            zoff += SP
            y2h = y2p.tile([IPC, PHWMAX * (H + 1)], f32, tag="y2")
            phase_state[h] = (zts, chunk_offs, y2h)

        def rhs(h, o, wdt):
            zts, chunk_offs, _ = phase_state[h]
            ci = max(i for i, c in enumerate(chunk_offs) if c <= o)
            return zts[ci][:, o - chunk_offs[ci]: o - chunk_offs[ci] + wdt]

        st = {}

        def s_mm0(i):
            h, (off, wa, wb) = flat[i]
            if h not in phase_state:
                start_phase(h)
            wt = wa + wb
            p0t = p0.tile([128, 1024], f32, tag="ps0")
            nc.tensor.matmul(p0t[:, 0:wa], a0r, rhs(h, off, wa),
                             start=True, stop=True)
            if wb:
                nc.tensor.matmul(p0t[:, wa:wt], a0r, rhs(h, off + wa, wb),
                                 start=True, stop=True)
            y0t = y0p.tile([128, 1024], f32r, tag="y0")
            evict(y0t[:, 0:wt], p0t[:, 0:wt], None, True)
            st[i] = [y0t]

        def s_mm1(i):
            h, (off, wa, wb) = flat[i]
            wt = wa + wb
            y0t = st[i][0]
            p1t = p12.tile([128, 1024], f32, tag="ps12")
            nc.tensor.matmul(p1t[:, 0:wa], w1r, y0t[:, 0:wa],
                             start=True, stop=True)
            if wb:
                nc.tensor.matmul(p1t[:, wa:wt], w1r, y0t[:, wa:wt],
                                 start=True, stop=True)
            y1t = y1p.tile([128, 1024], f32r, tag="y1")
            evict(y1t[:, 0:wt], p1t[:, 0:wt], b1ap, True)
            st[i].append(y1t)

        def s_mm2(i):
            h, (off, wa, wb) = flat[i]
            wt = wa + wb
            y1t = st[i][1]
            y2h = phase_state[h][2]
            p2t = p12.tile([128, 1024], f32, tag="ps12")
            nc.tensor.matmul(p2t[0:IPC, 0:wa], w2r, y1t[:, 0:wa],
                             start=True, stop=True)
            if wb:
                nc.tensor.matmul(p2t[0:IPC, wa:wt], w2r, y1t[:, wa:wt],
                                 start=True, stop=True)
            evict(y2h[:, PHW[h] + off: PHW[h] + off + wt], p2t[0:IPC, 0:wt],
                  b2ap, False)
            del st[i]
            if i + 1 >= len(flat) or flat[i + 1][0] != h:
                pending_ups.append((h, i))

        def emit_upsample(h):
            nonlocal prev_hin3
            WHp = PHW[h]
            y2h = phase_state[h][2]
            # front pad = duplicate of row 0 (makes the halo windows uniform)
            nc.gpsimd.tensor_copy(y2h[:, 0:WHp], y2h[:, WHp:2 * WHp])

            hint = hinp.tile([128, (RPB + 1) * PHWMAX], f32, tag="hin")
            ybase = y2h[:]
            hin3 = hint[:, 0:(RPB + 1) * WHp] \
                .rearrange("p (j c) -> p j c", j=RPB + 1)
            fxh = fxp.tile([128, (ORPB + 1) * (2 * PHWMAX)], f32, tag="fx")
            fx3 = fxh[:, 0:(ORPB + 1) * (2 * WHp)] \
                .rearrange("p (v c) -> p v c", v=ORPB + 1)

            # row-bands so the hin-DMA -> A/B -> C -> sigmoid -> out-DMA
            # chain pipelines against itself; the last phase gets three bands
            # and its final band's A/B run on ACT/DVE (idle during the tail)
            last_phase = (h == NPH - 1)
            bands = [(0, 7), (7, 13), (13, 18)] if last_phase else [(0, 10), (10, 18)]
            for bi, (j0, j1) in enumerate(bands):
                tailband = last_phase and bi == len(bands) - 1
                # re-partition DMA for this band (overlapping halo windows)
                bsrc = bass.AP(tensor=ybase.tensor,
                               offset=ybase.offset + j0 * WHp,
                               ap=[list(ybase.ap[0]), [RPB * WHp, BLK],
                                   [1, (j1 - j0) * WHp]])
                nc.scalar.dma_start(
                    hint[:, j0 * WHp:j1 * WHp].rearrange(
                        "p (j c) -> p j c", j=j1 - j0).rearrange(
                        "p j c -> p (j c)"), bsrc)
                hj = hin3[:, j0:j1, :]
                fe = fx3[:, 2 * j0:2 * j1 - 1:2, :]     # wout rows of band
                # pass A: odd out-cols, written as 2x values so every column
                # of a wout row shares one sigmoid scale class
                if tailband:
                    nc.scalar.mul(fe[:, :, 1:2 * WHp:2], hj, 2.0)
                else:
                    nc.gpsimd.tensor_scalar(fe[:, :, 1:2 * WHp:2], hj,
                                            2.0, None, Alu.mult)
                # out-col 0 (2x scale class)
                if h == 0:
                    nc.gpsimd.tensor_scalar(fe[:, :, 0:1], hj[:, :, 0:1],
                                            2.0, None, Alu.mult)
                else:
                    nc.vector.tensor_tensor(fe[:, :, 0:1],
                                            prev_hin3[:, j0:j1, -1:],
                                            hj[:, :, 0:1], Alu.add)
                # pass B: even out-cols = sums of adjacent in-cols
                bop = nc.vector if tailband else nc.gpsimd
                bop.tensor_tensor(fe[:, :, 2:2 * WHp - 1:2],
                                  hj[:, :, 0:WHp - 1], hj[:, :, 1:WHp],
                                  Alu.add)
                # pass C: odd fx rows = sums of adjacent wout rows
                # (pool for hidden phases, DVE for the exposed last phase)
                vo0, vo1 = max(1, 2 * j0 - 1), 2 * j1 - 2
                cop = nc.vector if last_phase else nc.gpsimd
                cop.tensor_tensor(fx3[:, vo0:vo1:2, :],
                                  fx3[:, vo0 - 1:vo1 - 1:2, :],
                                  fx3[:, vo0 + 1:vo1 + 1:2, :],
                                  Alu.add)
                # sigmoid, 0.5 factors folded into scale by row/col parity.
                # A band's top boundary wout row (v = 2*j1-2) is read pre-
                # sigmoid by the NEXT band's pass C, so its sigmoid + output
                # are deferred to that band.
                last_band = bi == len(bands) - 1
                ve0 = max(2, 2 * j0 - 2)
                ve1 = 2 * j1 - 1 if last_band else 2 * j1 - 3
                for (v0, v1), rowsc in (((ve0, ve1), 0.5), ((vo0, vo1), 0.25)):
                    ap_ = fx3[:, v0:v1:2, :]
                    nc.scalar.activation(ap_, ap_, Act.Sigmoid, scale=rowsc)
                vs0 = max(1, 2 * j0 - 2)
                vs1 = 2 * j1 - 1 if last_band else 2 * j1 - 2
                dst = outd[:, :, 2 * PHOFF[h]: 2 * (PHOFF[h] + WHp)] \
                    .rearrange("i (b v) c -> i b v c", b=BLK)[:, :, vs0 - 1:vs1 - 1, :]
                nc.sync.dma_start(dst, fx3[:, vs0:vs1, :])
            prev_hin3 = hin3

        pending_ups = []
        DELAY = 6
        for i in range(len(flat) + 2 + DELAY):
            if i < len(flat):
                s_mm0(i)
            if 0 <= i - 1 < len(flat):
                s_mm1(i - 1)
            if 0 <= i - 2 < len(flat):
                s_mm2(i - 2)
            while pending_ups and (i - 2 - pending_ups[0][1] >= DELAY
                                   or i - 2 >= len(flat)):
                emit_upsample(pending_ups.pop(0)[0])

    nc.compile()
    return nc


def _host_prep(mask_feats, mask_head_params, locations, im_inds, fpn_levels,
               sizes_of_interest):
    mask_feats = np.asarray(mask_feats, dtype=np.float32)
    params = np.asarray(mask_head_params, dtype=np.float32)
    locations = np.asarray(locations, dtype=np.float32)
    im_inds = np.asarray(im_inds).astype(np.int64)
    fpn_levels = np.asarray(fpn_levels).astype(np.int64)
    soi_tab = np.asarray(sizes_of_interest, dtype=np.float32)

    w0 = params[:, 0:80].reshape(N_INST, CH, CIN + 2)
    w1 = params[:, 80:144].reshape(N_INST, CH, CH)
    w2 = params[:, 144:152].reshape(N_INST, 1, CH)
    b0 = params[:, 152:160]
    b1 = params[:, 160:168]
    b2 = params[:, 168:169]

    soi = soi_tab[fpn_levels]                                    # (128,)
    alpha = -w0[:, :, 0] / soi[:, None]                          # (128, 8)
    beta = -w0[:, :, 1] / soi[:, None]
    c0 = b0 + (w0[:, :, 0] * locations[:, 0:1]
               + w0[:, :, 1] * locations[:, 1:2]) / soi[:, None]
    wfeat = w0[:, :, 2:]                                         # (128, 8, 8)

    stride = 8
    xs = np.arange(W, dtype=np.float32) * stride + stride // 2
    ys = np.arange(H, dtype=np.float32) * stride + stride // 2
    locs_x = np.tile(xs, H)
    locs_y = np.repeat(ys, W)
    z = np.concatenate([locs_x[None], locs_y[None],
                        np.ones((1, HW), np.float32),
                        mask_feats.reshape(N_IMG * CIN, HW)], axis=0)
    # reorder spatial into the uneven column-phase blocks
    z3 = z.reshape(K0, H, W)
    z = np.concatenate(
        [z3[:, :, PHOFF[q]:PHOFF[q] + PHW[q]].reshape(K0, H * PHW[q])
         for q in range(NPH)], axis=1)
    z = np.ascontiguousarray(z, dtype=np.float32)

    in_maps = []
    for c in range(N_CORES):
        a0 = np.zeros((K0, 128), np.float32)
        wpack = np.zeros((128, 146), np.float32)
        for i in range(IPC):
            gi = IPC * c + i
            for o in range(CH):
                m = CH * i + o
                a0[0, m] = alpha[gi, o]
                a0[1, m] = beta[gi, o]
                a0[2, m] = c0[gi, o]
                base = 3 + CIN * int(im_inds[gi])
                a0[base:base + CIN, m] = wfeat[gi, o, :]
                wpack[CH * i:CH * i + CH, m] = w1[gi, o, :]
                wpack[m, 144] = b1[gi, o]
            wpack[CH * i:CH * i + CH, 128 + i] = w2[gi, 0, :]
            wpack[i, 145] = b2[gi, 0]
        in_maps.append({
            "z_in": z,
            "a0t_in": np.ascontiguousarray(a0),
            "wpack_in": np.ascontiguousarray(wpack),
        })
    return in_maps


def kernel(mask_feats, mask_head_params, locations, im_inds, fpn_levels,
           sizes_of_interest, mask_feat_stride):
    global LAST_EXEC_TIME_NS
    assert int(mask_feat_stride) == 8, "kernel hardcodes mask_feat_stride=8"

    from concourse.bass_utils import run_bass_kernel_spmd

    in_maps = _host_prep(mask_feats, mask_head_params, locations, im_inds,
                         fpn_levels, sizes_of_interest)

    if "nc" not in _CACHE:
        _CACHE["nc"] = _build_program()
    nc = _CACHE["nc"]

    trace = bool(os.environ.get("BASS_TRACE"))
    res = run_bass_kernel_spmd(nc, in_maps, list(range(N_CORES)), trace=trace)
    LAST_EXEC_TIME_NS = res.exec_time_ns

    out = np.empty((N_INST, 1, OH, OW), np.float32)
    for c in range(N_CORES):
        out[IPC * c:IPC * (c + 1), 0] = np.asarray(res.results[c]["out"])
    return out

